# revision 12
# baseline (speedup 1.0000x reference)
"""MI-LSTM model fully on-device for Trainium2 (8 NeuronCores, batch-sharded).

Per core (BC=256 rows of B=2048):
  Stage 1: 21 shared-weight scalar-input LSTMs, fused as Q=21*256=5376 lanes.
    Layout: gate pre-activations z^T [256, Q] as two PSUM M-tiles
    [f|i] and [o|j]; sigma on ACT with per-partition bias/scale
    (tanh(j) = 2*sigma(2j)-1); state c and all elementwise in fp16 on DVE;
    one SBUF->SBUF DMA per step moves the i*tanh(j) product across the
    partition halves.  h feeds the next step's matmul rhs [h(0:64); x(64)].
  Stage 2 (pipelined per step): MI-LSTM x-projections as PSUM-accumulated
    matmuls over the 21 h-blocks, 3-way attention softmax via the
    sigma-ratio trick (exp(x) = s/(1-s), s = sigma(x); avoids an ACT
    table switch), fp32.
  Temporal attention + dense head on device; only [B,1] returned.
"""
import os
import sys

sys.path.insert(0, "/opt/trn_rl_repo")

import numpy as np

H = 64
NS = 10
NSER = 21
B = 2048
T = 50
NCORES = 8
BC = B // NCORES          # 256
Q = NSER * BC             # 5376
DP = NS * H

_CACHE = {}
_LAST_HW_NS = None


# ---------------------------------------------------------------------------
# Bass kernel builder
# ---------------------------------------------------------------------------

def _build():
    import concourse.bass as bass  # noqa: F401
    import concourse.tile as tile
    from concourse import bacc, mybir
    from contextlib import ExitStack

    f32 = mybir.dt.float32
    f32r = mybir.dt.float32r
    f16 = mybir.dt.float16
    AF = mybir.ActivationFunctionType
    ALU = mybir.AluOpType

    nc = bacc.Bacc("TRN2", target_bir_lowering=False, debug=False,
                   num_devices=NCORES)

    def din(name, shape, dt=f32):
        return nc.dram_tensor(name, shape, dt, kind="ExternalInput").ap()

    # --- DRAM inputs ---
    xt = din("xt", [T, Q], f16)
    w0 = din("w0", [65, 128], f16)       # lhsT [Kh;Kx] x gates [f|i]
    w1 = din("w1", [65, 128], f16)       # gates [o|j]
    bias0 = din("bias0", [128, 1])
    bias1 = din("bias1", [128, 1])
    scale1 = din("scale1", [128, 1])
    wy0 = din("wy0", [64, 128], f16)     # T0 x-side [Wi0_x|Wc0_x]
    wys3 = din("wys3", [64, 128], f16)   # T3 x-side [Wf_x|Wo_x]
    wp = din("wp", [64, 1280], f16)      # T1 x-side per P-series
    wn = din("wn", [64, 1280], f16)      # T2 x-side per N-series
    r0 = din("r0", [64, 128], f32r)       # recurrent h-side (f32r use)
    r1_ = din("r1", [64, 128], f32r)
    r2_ = din("r2", [64, 128], f32r)
    r3_ = din("r3", [64, 128], f32r)
    wa = din("wa", [64, 64], f32r)
    bt0 = din("bt0", [128, 1])
    bt1 = din("bt1", [128, 1])
    bt2 = din("bt2", [128, 1])
    bt3 = din("bt3", [128, 1])
    sc02 = din("sc02", [128, 1])
    onesel = din("onesel", [64, 9], f32r)     # u-select lhsTs
    ones1x64 = din("ones1x64", [1, 64], f32r)
    wtsel = din("wtsel", [64, T * T], f32r)   # e-select lhsTs [64,50] x 50
    id3 = din("id3", [3, 3])
    id50 = din("id50", [50, 50])
    id64 = din("id64", [64, 64])
    id64r = din("id64r", [64, 64], f32r)
    id128 = din("id128", [128, 128])
    wd1 = din("wd1", [64, 64], f32r)
    bd1 = din("bd1", [64, 1])
    wd2 = din("wd2", [64, 1], f32r)
    btv = din("btv", [50, 1])
    zeros64 = din("zeros64", [64, BC], f32r)
    consts = din("consts", [1, 2])  # [bt, bd2]
    out_d = nc.dram_tensor("out", [1, BC], f32, kind="ExternalOutput").ap()

    with tile.TileContext(nc) as tc:
        with ExitStack() as ctx:
            wpool = ctx.enter_context(tc.tile_pool(name="w", bufs=1))
            spool = ctx.enter_context(tc.tile_pool(name="st", bufs=1))
            lpool = ctx.enter_context(tc.tile_pool(name="lp", bufs=2))
            l1pool = ctx.enter_context(tc.tile_pool(name="l1", bufs=2))
            l2pool = ctx.enter_context(tc.tile_pool(name="l2", bufs=1))
            zpsum = ctx.enter_context(
                tc.tile_pool(name="zp", bufs=2, space="PSUM"))
            z2psum = ctx.enter_context(
                tc.tile_pool(name="z2p", bufs=1, space="PSUM"))
            epsum = ctx.enter_context(
                tc.tile_pool(name="ep", bufs=1, space="PSUM"))
            mpsum = ctx.enter_context(
                tc.tile_pool(name="mp", bufs=1, space="PSUM"))

            def sbload(ap_dram, shape, dt):
                t_ = wpool.tile(shape, dt, tag=ap_dram.tensor.name)
                nc.sync.dma_start(t_, ap_dram)
                return t_

            # --- load weights to SBUF ---
            w0s = sbload(w0, [65, 128], f16)
            w1s = sbload(w1, [65, 128], f16)
            b0s = sbload(bias0, [128, 1], f32)
            b1s = sbload(bias1, [128, 1], f32)
            s1s = sbload(scale1, [128, 1], f32)
            wy0s = sbload(wy0, [64, 128], f16)
            wys3s = sbload(wys3, [64, 128], f16)
            wps = sbload(wp, [64, 1280], f16)
            wns = sbload(wn, [64, 1280], f16)
            r0s = sbload(r0, [64, 128], f32r)
            r1s = sbload(r1_, [64, 128], f32r)
            r2s = sbload(r2_, [64, 128], f32r)
            r3s = sbload(r3_, [64, 128], f32r)
            was = sbload(wa, [64, 64], f32r)
            bt0s = sbload(bt0, [128, 1], f32)
            bt1s = sbload(bt1, [128, 1], f32)
            bt2s = sbload(bt2, [128, 1], f32)
            bt3s = sbload(bt3, [128, 1], f32)
            sc02s = sbload(sc02, [128, 1], f32)
            onesels = sbload(onesel, [64, 9], f32r)
            ones64s = sbload(ones1x64, [1, 64], f32r)
            wtsels = sbload(wtsel, [64, T * T], f32r)
            id3s = sbload(id3, [3, 3], f32)
            id50s = sbload(id50, [50, 50], f32)
            id64s = sbload(id64, [64, 64], f32)
            id64rs = sbload(id64r, [64, 64], f32r)
            id128s = sbload(id128, [128, 128], f32)
            wd1s = sbload(wd1, [64, 64], f32r)
            bd1s = sbload(bd1, [64, 1], f32)
            wd2s = sbload(wd2, [64, 1], f32r)
            btvs = sbload(btv, [50, 1], f32)
            constss = sbload(consts, [1, 2], f32)

            # --- persistent state ---
            xh_a = spool.tile([65, Q], f16)
            xh_b = spool.tile([65, Q], f16)
            cst = spool.tile([64, Q], f16)
            c2 = spool.tile([64, BC], f32r)
            h2z = spool.tile([64, BC], f32r)
            H2T = spool.tile([128, 2, T, 64], f32)
            e_ps = epsum.tile([50, BC], f32)

            nc.vector.memset(xh_a[0:64, :], 0.0)
            nc.vector.memset(cst[:, :], 0.0)
            nc.sync.dma_start(c2, zeros64)
            nc.sync.dma_start(h2z, zeros64)

            MM = nc.tensor.matmul
            ACT = nc.scalar.activation
            TT = nc.vector.tensor_tensor
            h2prev = h2z

            for t in range(T):
                xh = xh_a if t % 2 == 0 else xh_b
                xh_n = xh_b if t % 2 == 0 else xh_a
                # x row for this step
                nc.sync.dma_start(xh[64:65, :], xt[t:t + 1, :])

                zsb0 = lpool.tile([128, Q], f16, tag="zsb0")
                zsb1 = lpool.tile([128, Q], f16, tag="zsb1")
                # --- stage1 z matmuls + sigma ---
                for mt, (wmt, zsb, bvec, svec) in enumerate(
                        ((w0s, zsb0, b0s, None), (w1s, zsb1, b1s, s1s))):
                    for c0 in range(0, Q, 1024):
                        cw = min(1024, Q - c0)
                        zp = zpsum.tile([128, 1024], f32, tag="zp")
                        for wo in range(0, cw, 512):
                            ww = min(512, cw - wo)
                            MM(zp[:, wo:wo + ww], wmt,
                               xh[:, c0 + wo:c0 + wo + ww],
                               start=True, stop=True)
                        ACT(zsb[:, c0:c0 + cw], zp[:, 0:cw], AF.Sigmoid,
                            bias=bvec, scale=(svec if svec is not None
                                              else 1.0))
                # tanh(j) = 2*sigma(2j) - 1  (in-place on zsb1 hi rows)
                nc.vector.tensor_scalar(zsb1[64:128, :], zsb1[64:128, :],
                                        2.0, -1.0, ALU.mult, ALU.add)
                # p = sigma(i) * tanh(j)   (hi partitions)
                ppair = l2pool.tile([128, Q], f16, tag="pp")
                TT(ppair[64:128, :], zsb0[64:128, :], zsb1[64:128, :],
                   ALU.mult)
                # move p across the partition boundary
                nc.sync.dma_start(ppair[0:64, :], ppair[64:128, :])
                # c' = sigma(f)*c + p
                cfb = l2pool.tile([64, Q], f16, tag="cf")
                TT(cfb, zsb0[0:64, :], cst, ALU.mult)
                TT(cst, cfb, ppair[0:64, :], ALU.add)
                # h = sigma(o) * tanh(c)
                tcb = l2pool.tile([64, Q], f16, tag="tc")
                ACT(tcb, cst, AF.Tanh)
                TT(xh_n[0:64, :], zsb1[0:64, :], tcb, ALU.mult)

                # ================= stage 2 (step t) =================
                h = xh_n[0:64]
                h2r = h2prev
                z2a = z2psum.tile([128, 512], f32, tag="z2a")
                z2b = z2psum.tile([128, 512], f32, tag="z2b")
                # T0 = [i0|C0], T1 = [i1|C1] in z2a; T2 = [i2|C2], T3 = [f|o]
                MM(z2a[:, 0:BC], wy0s, h[:, 0:BC], start=True, stop=False)
                MM(z2a[:, 0:BC], r0s, h2r, start=False, stop=True)
                for s in range(NS):
                    MM(z2a[:, BC:2 * BC], wps[:, s * 128:(s + 1) * 128],
                       h[:, (1 + s) * BC:(2 + s) * BC],
                       start=(s == 0), stop=False)
                MM(z2a[:, BC:2 * BC], r1s, h2r, start=False, stop=True)
                for s in range(NS):
                    MM(z2b[:, 0:BC], wns[:, s * 128:(s + 1) * 128],
                       h[:, (11 + s) * BC:(12 + s) * BC],
                       start=(s == 0), stop=False)
                MM(z2b[:, 0:BC], r2s, h2r, start=False, stop=True)
                MM(z2b[:, BC:2 * BC], wys3s, h[:, 0:BC], start=True,
                   stop=False)
                MM(z2b[:, BC:2 * BC], r3s, h2r, start=False, stop=True)
                # g = tanh(c2 @ Wa)
                g_ps = mpsum.tile([64, BC], f32, tag="sm")
                MM(g_ps, was, c2, start=True, stop=True)
                gsb = l1pool.tile([64, BC], f32, tag="g")
                ACT(gsb, g_ps, AF.Tanh)
                # sigma passes  (C-gates: sigma(2x+2b) -> tanh fix later)
                z2sb = l2pool.tile([128, 1024], f32, tag="z2sb")
                ACT(z2sb[:, 0:256], z2a[:, 0:BC], AF.Sigmoid,
                    bias=bt0s, scale=sc02s)
                ACT(z2sb[:, 256:512], z2a[:, BC:2 * BC], AF.Sigmoid,
                    bias=bt1s, scale=sc02s)
                ACT(z2sb[:, 512:768], z2b[:, 0:BC], AF.Sigmoid,
                    bias=bt2s, scale=sc02s)
                ACT(z2sb[:, 768:1024], z2b[:, BC:2 * BC], AF.Sigmoid,
                    bias=bt3s)
                # move [C-gates sigma | sigma_o] to low partitions
                mv = l2pool.tile([64, 1024], f32, tag="mv")
                nc.sync.dma_start(mv, z2sb[64:128, :])
                nc.vector.tensor_scalar(mv[:, 0:768], mv[:, 0:768],
                                        2.0, -1.0, ALU.mult, ALU.add)
                # l_k = sigma(i_k) * tanh(C_k)
                lall = l2pool.tile([64, 768], f32, tag="lall")
                TT(lall, z2sb[0:64, 0:768], mv[:, 0:768], ALU.mult)
                # u_k = sum_h l_k * g
                lg = l2pool.tile([64, 768], f32r, tag="lg")
                for k in range(3):
                    TT(lg[:, k * BC:(k + 1) * BC],
                       lall[:, k * BC:(k + 1) * BC], gsb, ALU.mult)
                u_ps = mpsum.tile([3, BC], f32, tag="sm")
                for k in range(3):
                    MM(u_ps, onesels[:, 3 * k:3 * (k + 1)],
                       lg[:, k * BC:(k + 1) * BC],
                       start=(k == 0), stop=(k == 2))
                usb = l1pool.tile([3, BC], f32, tag="usb")
                nc.vector.tensor_copy(usb, u_ps)
                # transpose u -> [128, 2, 3]
                uT = l1pool.tile([128, 2, 3], f32, tag="uT")
                for bh in range(2):
                    tp = mpsum.tile([128, 3], f32, tag="sm")
                    nc.tensor.transpose(tp, usb[:, bh * 128:(bh + 1) * 128],
                                        id3s)
                    nc.vector.tensor_copy(uT[:, bh, :], tp)
                # softmax over k via sigma-ratio with max-subtract
                um = l1pool.tile([128, 2], f32, tag="um")
                nc.vector.tensor_reduce(um, uT[:, :, :], mybir.AxisListType.X,
                                        ALU.max)
                for k in range(3):
                    TT(uT[:, :, k], uT[:, :, k], um, ALU.subtract)
                ssb = l1pool.tile([128, 2, 3], f32, tag="ssb")
                ACT(ssb[:, :, :], uT[:, :, :], AF.Sigmoid)
                q1 = l1pool.tile([128, 2, 3], f32, tag="q1")
                nc.vector.tensor_scalar(q1[:, :, :], ssb[:, :, :], -1.0, 1.0,
                                        ALU.mult, ALU.add)
                nc.vector.reciprocal(q1[:, :, :], q1[:, :, :])
                wk = l1pool.tile([128, 2, 3], f32, tag="wk")
                TT(wk[:, :, :], ssb[:, :, :], q1[:, :, :], ALU.mult)
                ws = l1pool.tile([128, 2], f32, tag="ws")
                nc.vector.tensor_reduce(ws, wk[:, :, :], mybir.AxisListType.X,
                                        ALU.add)
                nc.vector.reciprocal(ws, ws)
                for k in range(3):
                    TT(wk[:, :, k], wk[:, :, k], ws, ALU.mult)
                # transpose a back -> [3, 256]
                asb = l1pool.tile([3, BC], f32r, tag="asb")
                for bh in range(2):
                    tp = mpsum.tile([3, 128], f32, tag="sm")
                    nc.tensor.transpose(tp, wk[:, bh, :], id128s)
                    nc.vector.tensor_copy(asb[:, bh * 128:(bh + 1) * 128], tp)
                # L = sum_k a_k * l_k  (broadcast a over 64 partitions via PE)
                a3 = l1pool.tile([1, 3 * BC], f32r, tag="a3")
                nc.sync.dma_start(a3, asb)   # [3,256] -> [1,768] row-major
                Lb = l1pool.tile([64, BC], f32, tag="Lb")
                Lt = l1pool.tile([64, BC], f32, tag="Lt")
                for k in range(3):
                    ab_ps = mpsum.tile([64, BC], f32, tag="sm")
                    MM(ab_ps, ones64s, a3[0:1, k * BC:(k + 1) * BC],
                       start=True, stop=True)
                    if k == 0:
                        TT(Lb, ab_ps, lall[:, 0:BC], ALU.mult)
                    else:
                        TT(Lt, ab_ps, lall[:, k * BC:(k + 1) * BC], ALU.mult)
                        TT(Lb, Lb, Lt, ALU.add)
                # c2' = sigma(f)*c2 + L ; h2 = sigma(o)*tanh(c2')
                t2 = l1pool.tile([64, BC], f32, tag="t2")
                TT(t2, z2sb[0:64, 768:1024], c2, ALU.mult)
                TT(c2, t2, Lb, ALU.add)
                tc2 = l1pool.tile([64, BC], f32, tag="tc2")
                ACT(tc2, c2, AF.Tanh)
                h2 = l1pool.tile([64, BC], f32r, tag="h2")
                TT(h2, mv[:, 768:1024], tc2, ALU.mult)
                # e accumulation + transposed H2 store
                MM(e_ps, wtsels[:, t * T:(t + 1) * T], h2,
                   start=(t == 0), stop=(t == T - 1))
                for bh in range(2):
                    tp = mpsum.tile([128, 64], f32, tag="sm")
                    nc.tensor.transpose(tp.bitcast(f32r),
                                        h2[:, bh * 128:(bh + 1) * 128],
                                        id64rs)
                    nc.vector.tensor_copy(H2T[:, bh, t, :], tp)
                h2prev = h2

            # ======================= head =======================
            esb = spool.tile([50, BC], f32)
            ACT(esb, e_ps, AF.Tanh, bias=btvs)
            eT = spool.tile([128, 2, 50], f32)
            for bh in range(2):
                tp = mpsum.tile([128, 50], f32, tag="sm")
                nc.tensor.transpose(tp, esb[:, bh * 128:(bh + 1) * 128],
                                    id50s)
                nc.vector.tensor_copy(eT[:, bh, :], tp)
            sh = spool.tile([128, 2, 50], f32)
            ACT(sh[:, :, :], eT[:, :, :], AF.Sigmoid)
            qh = spool.tile([128, 2, 50], f32)
            nc.vector.tensor_scalar(qh[:, :, :], sh[:, :, :], -1.0, 1.0,
                                    ALU.mult, ALU.add)
            nc.vector.reciprocal(qh[:, :, :], qh[:, :, :])
            whd = spool.tile([128, 2, 50], f32)
            TT(whd[:, :, :], sh[:, :, :], qh[:, :, :], ALU.mult)
            wsh = spool.tile([128, 2], f32)
            nc.vector.tensor_reduce(wsh, whd[:, :, :], mybir.AxisListType.X,
                                    ALU.add)
            nc.vector.reciprocal(wsh, wsh)
            # ctx: scale H2T rows by w and tree-reduce over t
            for bh in range(2):
                for tt in range(T):
                    nc.vector.tensor_scalar(
                        H2T[:, bh, tt, :], H2T[:, bh, tt, :],
                        whd[:, bh, tt:tt + 1], None, ALU.mult)
            # tree reduce 50 -> 1 (in place over t axis)
            n = T
            while n > 1:
                half = n // 2
                for bh in range(2):
                    TT(H2T[:, bh, 0:half, :], H2T[:, bh, 0:half, :],
                       H2T[:, bh, half:2 * half, :], ALU.add)
                if n % 2 == 1:
                    for bh in range(2):
                        TT(H2T[:, bh, 0, :], H2T[:, bh, 0, :],
                           H2T[:, bh, n - 1, :], ALU.add)
                n = half
            ctxT = spool.tile([128, 2, 64], f32)
            for bh in range(2):
                nc.vector.tensor_scalar(ctxT[:, bh, :], H2T[:, bh, 0, :],
                                        wsh[:, bh:bh + 1], None, ALU.mult)
            ctxs = spool.tile([64, BC], f32r)
            for bh in range(2):
                tp = mpsum.tile([64, 128], f32, tag="sm")
                nc.tensor.transpose(tp, ctxT[:, bh, :], id128s)
                nc.vector.tensor_copy(ctxs[:, bh * 128:(bh + 1) * 128], tp)
            r1_ps = mpsum.tile([64, BC], f32, tag="sm")
            MM(r1_ps, wd1s, ctxs, start=True, stop=True)
            r1sb = spool.tile([64, BC], f32r)
            ACT(r1sb, r1_ps, AF.Relu, bias=bd1s)
            o_ps = mpsum.tile([1, BC], f32, tag="sm")
            MM(o_ps, wd2s, r1sb, start=True, stop=True)
            osb = spool.tile([1, BC], f32)
            ACT(osb, o_ps, AF.Identity, bias=constss[0:1, 1:2])
            nc.sync.dma_start(out_d, osb)

    nc.compile()
    return nc


# ---------------------------------------------------------------------------
# Host-side input prep
# ---------------------------------------------------------------------------

def _prep_weights(K1, b1, Wc0, bc0, Wc1, bc1, Wc2, bc2,
                  Wi0, bi0, Wi1, bi1, Wi2, bi2, Wf, bf, Wo, bo, Wa,
                  Wt, bt, Wd1, bd1, Wd2, bd2):
    f32, f16 = np.float32, np.float16
    K1 = np.asarray(K1, f32); b1 = np.asarray(b1, f32)
    d = {}
    # stage1 lhsTs: rows = [Kh(64); Kx(1)]; col order tiles [f|i], [o|j]
    kr = np.concatenate([K1[1:65], K1[0:1]], axis=0)  # [65, 256]
    d["w0"] = np.concatenate([kr[:, 128:192], kr[:, 0:64]], axis=1).astype(f16)
    d["w1"] = np.concatenate([kr[:, 192:256], kr[:, 64:128]],
                             axis=1).astype(f16)
    d["bias0"] = np.concatenate([b1[128:192] + 1.0,
                                 b1[0:64]]).reshape(128, 1).astype(f32)
    d["bias1"] = np.concatenate([b1[192:256],
                                 2.0 * b1[64:128]]).reshape(128, 1).astype(f32)
    d["scale1"] = np.concatenate([np.ones(64), 2.0 * np.ones(64)]
                                 ).reshape(128, 1).astype(f32)
    # stage2
    Wi0, Wc0 = np.asarray(Wi0, f32), np.asarray(Wc0, f32)
    Wi1, Wc1 = np.asarray(Wi1, f32), np.asarray(Wc1, f32)
    Wi2, Wc2 = np.asarray(Wi2, f32), np.asarray(Wc2, f32)
    Wf_, Wo_ = np.asarray(Wf, f32), np.asarray(Wo, f32)
    d["wy0"] = np.concatenate([Wi0[:64], Wc0[:64]], axis=1).astype(f16)
    d["wys3"] = np.concatenate([Wf_[:64], Wo_[:64]], axis=1).astype(f16)
    wp = np.zeros((64, 1280), f16)
    wn = np.zeros((64, 1280), f16)
    for s in range(NS):
        wp[:, s * 128:s * 128 + 64] = Wi1[s * 64:(s + 1) * 64]
        wp[:, s * 128 + 64:(s + 1) * 128] = Wc1[s * 64:(s + 1) * 64]
        wn[:, s * 128:s * 128 + 64] = Wi2[s * 64:(s + 1) * 64]
        wn[:, s * 128 + 64:(s + 1) * 128] = Wc2[s * 64:(s + 1) * 64]
    d["wp"], d["wn"] = wp, wn
    d["r0"] = np.concatenate([Wi0[64:], Wc0[64:]], axis=1).astype(f32)
    d["r1"] = np.concatenate([Wi1[640:], Wc1[640:]], axis=1).astype(f32)
    d["r2"] = np.concatenate([Wi2[640:], Wc2[640:]], axis=1).astype(f32)
    d["r3"] = np.concatenate([Wf_[64:], Wo_[64:]], axis=1).astype(f32)
    d["wa"] = np.asarray(Wa, f32)
    bi0, bc0 = np.asarray(bi0, f32), np.asarray(bc0, f32)
    bi1, bc1 = np.asarray(bi1, f32), np.asarray(bc1, f32)
    bi2, bc2 = np.asarray(bi2, f32), np.asarray(bc2, f32)
    bf_, bo_ = np.asarray(bf, f32), np.asarray(bo, f32)
    d["bt0"] = np.concatenate([bi0, 2 * bc0]).reshape(128, 1).astype(f32)
    d["bt1"] = np.concatenate([bi1, 2 * bc1]).reshape(128, 1).astype(f32)
    d["bt2"] = np.concatenate([bi2, 2 * bc2]).reshape(128, 1).astype(f32)
    d["bt3"] = np.concatenate([bf_, bo_]).reshape(128, 1).astype(f32)
    d["sc02"] = d["scale1"]
    onesel = np.zeros((64, 9), f32)
    for k in range(3):
        onesel[:, 3 * k + k] = 1.0
    d["onesel"] = onesel
    d["ones1x64"] = np.ones((1, 64), f32)
    Wt = np.asarray(Wt, f32)
    wtsel = np.zeros((64, T * T), f32)
    for t_ in range(T):
        wtsel[:, t_ * T + t_] = Wt[:, 0]
    d["wtsel"] = wtsel
    d["id3"] = np.eye(3, dtype=f32)
    d["id50"] = np.eye(50, dtype=f32)
    d["id64"] = np.eye(64, dtype=f32)
    d["id64r"] = np.eye(64, dtype=f32)
    d["id128"] = np.eye(128, dtype=f32)
    d["wd1"] = np.asarray(Wd1, f32)
    d["bd1"] = np.asarray(bd1, f32).reshape(64, 1)
    d["wd2"] = np.asarray(Wd2, f32)
    d["zeros64"] = np.zeros((64, BC), f32)
    d["btv"] = np.full((50, 1), float(np.asarray(bt).reshape(-1)[0]), f32)
    d["consts"] = np.array([[float(np.asarray(bt).reshape(-1)[0]),
                             float(np.asarray(bd2).reshape(-1)[0])]], f32)
    return d


def _run_bass(Y, P, N, wd):
    global _LAST_HW_NS
    from concourse.bass_utils import run_bass_kernel_spmd
    if "nc" not in _CACHE:
        _CACHE["nc"] = _build()
    nc = _CACHE["nc"]
    series = np.concatenate([np.asarray(Y, np.float32),
                             np.asarray(P, np.float32),
                             np.asarray(N, np.float32)], axis=2)  # [B,T,21]
    in_maps = []
    for c in range(NCORES):
        blk = series[c * BC:(c + 1) * BC]            # [BC,T,21]
        xtc = np.ascontiguousarray(
            np.transpose(blk, (1, 2, 0)).reshape(T, Q)).astype(np.float16)
        m = {"xt": xtc}
        m.update(wd)
        in_maps.append(m)
    trace = os.environ.get("KERNEL_TRACE", "0") == "1"
    if trace:
        try:
            import antenv.axon_hooks  # noqa: F401
        except ImportError:
            trace = False
    kw = {}
    if trace:
        kw = {"trace": True, "trace_cores": [0]}
    res = run_bass_kernel_spmd(nc, in_maps, list(range(NCORES)), **kw)
    if getattr(res, "exec_time_ns", None) is not None:
        _LAST_HW_NS = res.exec_time_ns
    outs = [res.results[c]["out"].reshape(BC, 1) for c in range(NCORES)]
    return np.concatenate(outs, axis=0).astype(np.float32)


# ---------------------------------------------------------------------------
# Host fallback (numpy, used only if the device path fails)
# ---------------------------------------------------------------------------

def _host_ref(Y, P, N, K1, b1, Wc0, bc0, Wc1, bc1, Wc2, bc2,
              Wi0, bi0, Wi1, bi1, Wi2, bi2, Wf, bf, Wo, bo, Wa,
              Wt, bt, Wd1, bd1, Wd2, bd2):
    f32 = np.float32
    sig = lambda x: 1.0 / (1.0 + np.exp(-x))
    series = np.concatenate([Y, P, N], axis=2)
    x21 = np.moveaxis(series, 2, 0)[..., None].reshape(NSER * B, T, 1)
    Kx, Kh = K1[0], K1[1:]
    h = np.zeros((NSER * B, H), f32); c = np.zeros((NSER * B, H), f32)
    hs = np.empty((NSER * B, T, H), f32)
    for t in range(T):
        z = x21[:, t, 0:1] * Kx[None, :] + h @ Kh + b1
        i, j, f, o = np.split(z, 4, axis=1)
        c = sig(f + 1.0) * c + sig(i) * np.tanh(j)
        h = sig(o) * np.tanh(c)
        hs[:, t] = h
    hs = hs.reshape(NSER, B, T, H)
    Y1 = hs[0]
    pres = np.moveaxis(hs[1:1 + NS], 0, 2).reshape(B, T, NS * H)
    nres = np.moveaxis(hs[1 + NS:], 0, 2).reshape(B, T, NS * H)
    X = np.concatenate([Y1, pres, nres], axis=2)
    h2 = np.zeros((B, H), f32); c2 = np.zeros((B, H), f32)
    H2 = np.empty((B, T, H), f32)
    for t in range(T):
        xt_ = X[:, t]
        xY, xP, xN = xt_[:, :H], xt_[:, H:H + DP], xt_[:, H + DP:]
        def br(x_, Wi, bi_, Wc, bc_):
            z_ = np.concatenate([x_, h2], axis=1)
            return sig(z_ @ Wi + bi_) * np.tanh(z_ @ Wc + bc_)
        l0 = br(xY, Wi0, bi0, Wc0, bc0)
        l1 = br(xP, Wi1, bi1, Wc1, bc1)
        l2 = br(xN, Wi2, bi2, Wc2, bc2)
        g = np.tanh(c2 @ Wa)
        u = np.stack([(l0 * g).sum(1), (l1 * g).sum(1), (l2 * g).sum(1)], 1)
        u = u - u.max(1, keepdims=True)
        e_ = np.exp(u); a = e_ / e_.sum(1, keepdims=True)
        L = a[:, 0:1] * l0 + a[:, 1:2] * l1 + a[:, 2:3] * l2
        zY = np.concatenate([xY, h2], axis=1)
        c2 = sig(zY @ Wf + bf) * c2 + L
        h2 = sig(zY @ Wo + bo) * np.tanh(c2)
        H2[:, t] = h2
    e = np.tanh(H2 @ Wt + bt)
    e = e - e.max(1, keepdims=True)
    bta = np.exp(e); bta = bta / bta.sum(1, keepdims=True)
    ctx = (bta * H2).sum(1)
    r1 = np.maximum(ctx @ Wd1 + bd1, 0.0)
    return (r1 @ Wd2 + bd2).astype(f32)


# ---------------------------------------------------------------------------
# Entry point
# ---------------------------------------------------------------------------

def kernel(Y, P, N, K1, b1, Wc0, bc0, Wc1, bc1, Wc2, bc2,
           Wi0, bi0, Wi1, bi1, Wi2, bi2, Wf, bf, Wo, bo, Wa,
           Wt, bt, Wd1, bd1, Wd2, bd2):
    args = dict(K1=K1, b1=b1, Wc0=Wc0, bc0=bc0, Wc1=Wc1, bc1=bc1, Wc2=Wc2,
                bc2=bc2, Wi0=Wi0, bi0=bi0, Wi1=Wi1, bi1=bi1, Wi2=Wi2,
                bi2=bi2, Wf=Wf, bf=bf, Wo=Wo, bo=bo, Wa=Wa, Wt=Wt, bt=bt,
                Wd1=Wd1, bd1=bd1, Wd2=Wd2, bd2=bd2)
    if os.environ.get("KERNEL_NO_BASS", "0") != "1":
        try:
            wd = _prep_weights(**args)
            return _run_bass(Y, P, N, wd)
        except Exception as e:  # noqa: BLE001
            import traceback
            traceback.print_exc()
            sys.stderr.write(f"bass path failed ({e!r}); host fallback\n")
    f32 = np.float32
    return _host_ref(np.asarray(Y, f32), np.asarray(P, f32),
                     np.asarray(N, f32),
                     *[np.asarray(args[k], f32) for k in
                       ("K1", "b1", "Wc0", "bc0", "Wc1", "bc1", "Wc2", "bc2",
                        "Wi0", "bi0", "Wi1", "bi1", "Wi2", "bi2", "Wf", "bf",
                        "Wo", "bo", "Wa", "Wt", "bt", "Wd1", "bd1", "Wd2",
                        "bd2")])


# revision 13
# speedup vs baseline: 1.2064x; 1.2064x over previous
"""MI-LSTM model fully on-device for Trainium2 (8 NeuronCores, batch-sharded).

Per core (BC=256 rows of B=2048):
  Stage 1: 21 shared-weight scalar-input LSTMs, fused as Q=21*256=5376 lanes.
    Layout: gate pre-activations z^T [256, Q] as two PSUM M-tiles
    [f|i] and [o|j]; sigma on ACT with per-partition bias/scale
    (tanh(j) = 2*sigma(2j)-1); state c and all elementwise in fp16 on DVE;
    one SBUF->SBUF DMA per step moves the i*tanh(j) product across the
    partition halves.  h feeds the next step's matmul rhs [h(0:64); x(64)].
  Stage 2 (pipelined per step): MI-LSTM x-projections as PSUM-accumulated
    matmuls over the 21 h-blocks, 3-way attention softmax via the
    sigma-ratio trick (exp(x) = s/(1-s), s = sigma(x); avoids an ACT
    table switch), fp32.
  Temporal attention + dense head on device; only [B,1] returned.
"""
import os
import sys

sys.path.insert(0, "/opt/trn_rl_repo")

import numpy as np

H = 64
NS = 10
NSER = 21
B = 2048
T = 50
NCORES = 8
BC = B // NCORES          # 256
Q = NSER * BC             # 5376
DP = NS * H

_CACHE = {}
_LAST_HW_NS = None


# ---------------------------------------------------------------------------
# Bass kernel builder
# ---------------------------------------------------------------------------

def _build():
    import concourse.bass as bass  # noqa: F401
    import concourse.tile as tile
    from concourse import bacc, mybir
    from contextlib import ExitStack

    f32 = mybir.dt.float32
    f32r = mybir.dt.float32r
    f16 = mybir.dt.float16
    AF = mybir.ActivationFunctionType
    ALU = mybir.AluOpType

    nc = bacc.Bacc("TRN2", target_bir_lowering=False, debug=False,
                   num_devices=NCORES)

    def din(name, shape, dt=f32):
        return nc.dram_tensor(name, shape, dt, kind="ExternalInput").ap()

    # --- DRAM inputs ---
    xt = din("xt", [T, Q], f16)
    w0 = din("w0", [65, 128], f16)       # lhsT [Kh;Kx] x gates [f|i]
    w1 = din("w1", [65, 128], f16)       # gates [o|j]
    bias0 = din("bias0", [128, 1])
    bias1 = din("bias1", [128, 1])
    scale1 = din("scale1", [128, 1])
    wy0 = din("wy0", [64, 128], f16)     # T0 x-side [Wi0_x|Wc0_x]
    wys3 = din("wys3", [64, 128], f16)   # T3 x-side [Wf_x|Wo_x]
    wp = din("wp", [64, 1280], f16)      # T1 x-side per P-series
    wn = din("wn", [64, 1280], f16)      # T2 x-side per N-series
    r0 = din("r0", [64, 128], f32r)       # recurrent h-side (f32r use)
    r1_ = din("r1", [64, 128], f32r)
    r2_ = din("r2", [64, 128], f32r)
    r3_ = din("r3", [64, 128], f32r)
    wa = din("wa", [64, 64], f32r)
    bt0 = din("bt0", [128, 1])
    bt1 = din("bt1", [128, 1])
    bt2 = din("bt2", [128, 1])
    bt3 = din("bt3", [128, 1])
    sc02 = din("sc02", [128, 1])
    onesel = din("onesel", [64, 9], f32r)     # u-select lhsTs
    ones1x64 = din("ones1x64", [1, 64], f32r)
    wtsel = din("wtsel", [64, T * T], f32r)   # e-select lhsTs [64,50] x 50
    id3 = din("id3", [3, 3])
    id50 = din("id50", [50, 50])
    id64 = din("id64", [64, 64])
    id64r = din("id64r", [64, 64], f32r)
    id128 = din("id128", [128, 128])
    wd1 = din("wd1", [64, 64], f32r)
    bd1 = din("bd1", [64, 1])
    wd2 = din("wd2", [64, 1], f32r)
    btv = din("btv", [50, 1])
    zeros64 = din("zeros64", [64, BC], f32r)
    consts = din("consts", [1, 2])  # [bt, bd2]
    out_d = nc.dram_tensor("out", [1, BC], f32, kind="ExternalOutput").ap()

    with tile.TileContext(nc) as tc:
        with ExitStack() as ctx:
            wpool = ctx.enter_context(tc.tile_pool(name="w", bufs=1))
            spool = ctx.enter_context(tc.tile_pool(name="st", bufs=1))
            lpool = ctx.enter_context(tc.tile_pool(name="lp", bufs=2))
            l1pool = ctx.enter_context(tc.tile_pool(name="l1", bufs=2))
            l2pool = ctx.enter_context(tc.tile_pool(name="l2", bufs=1))
            zpsum = ctx.enter_context(
                tc.tile_pool(name="zp", bufs=2, space="PSUM"))
            z2psum = ctx.enter_context(
                tc.tile_pool(name="z2p", bufs=1, space="PSUM"))
            epsum = ctx.enter_context(
                tc.tile_pool(name="ep", bufs=1, space="PSUM"))
            mpsum = ctx.enter_context(
                tc.tile_pool(name="mp", bufs=1, space="PSUM"))

            def sbload(ap_dram, shape, dt):
                t_ = wpool.tile(shape, dt, tag=ap_dram.tensor.name)
                nc.sync.dma_start(t_, ap_dram)
                return t_

            # --- load weights to SBUF ---
            w0s = sbload(w0, [65, 128], f16)
            w1s = sbload(w1, [65, 128], f16)
            b0s = sbload(bias0, [128, 1], f32)
            b1s = sbload(bias1, [128, 1], f32)
            s1s = sbload(scale1, [128, 1], f32)
            wy0s = sbload(wy0, [64, 128], f16)
            wys3s = sbload(wys3, [64, 128], f16)
            wps = sbload(wp, [64, 1280], f16)
            wns = sbload(wn, [64, 1280], f16)
            r0s = sbload(r0, [64, 128], f32r)
            r1s = sbload(r1_, [64, 128], f32r)
            r2s = sbload(r2_, [64, 128], f32r)
            r3s = sbload(r3_, [64, 128], f32r)
            was = sbload(wa, [64, 64], f32r)
            bt0s = sbload(bt0, [128, 1], f32)
            bt1s = sbload(bt1, [128, 1], f32)
            bt2s = sbload(bt2, [128, 1], f32)
            bt3s = sbload(bt3, [128, 1], f32)
            sc02s = sbload(sc02, [128, 1], f32)
            onesels = sbload(onesel, [64, 9], f32r)
            ones64s = sbload(ones1x64, [1, 64], f32r)
            wtsels = sbload(wtsel, [64, T * T], f32r)
            id3s = sbload(id3, [3, 3], f32)
            id50s = sbload(id50, [50, 50], f32)
            id64s = sbload(id64, [64, 64], f32)
            id64rs = sbload(id64r, [64, 64], f32r)
            id128s = sbload(id128, [128, 128], f32)
            wd1s = sbload(wd1, [64, 64], f32r)
            bd1s = sbload(bd1, [64, 1], f32)
            wd2s = sbload(wd2, [64, 1], f32r)
            btvs = sbload(btv, [50, 1], f32)
            constss = sbload(consts, [1, 2], f32)

            # --- persistent state ---
            xh_a = spool.tile([65, Q], f16)
            xh_b = spool.tile([65, Q], f16)
            cst = spool.tile([64, Q], f16)
            c2 = spool.tile([64, BC], f32r)
            h2z = spool.tile([64, BC], f32r)
            H2T = spool.tile([128, 2, T, 64], f32)
            e_ps = epsum.tile([50, BC], f32)

            nc.vector.memset(xh_a[0:64, :], 0.0)
            nc.vector.memset(cst[:, :], 0.0)
            nc.sync.dma_start(c2, zeros64)
            nc.sync.dma_start(h2z, zeros64)

            MM = nc.tensor.matmul
            ACT = nc.scalar.activation
            TT = nc.vector.tensor_tensor
            h2prev = h2z

            for t in range(T):
                xh = xh_a if t % 2 == 0 else xh_b
                xh_n = xh_b if t % 2 == 0 else xh_a
                # x row for this step
                nc.sync.dma_start(xh[64:65, :], xt[t:t + 1, :])

                zsb0 = lpool.tile([128, Q], f16, tag="zsb0")
                zsb1 = lpool.tile([128, Q], f16, tag="zsb1")
                ppair = l2pool.tile([128, Q], f16, tag="pp")
                cfb = l2pool.tile([64, Q], f16, tag="cf")
                tcb = l2pool.tile([64, Q], f16, tag="tc")
                # --- stage1: z matmuls + sigma + c/h chain, chunked so
                # successive steps pipeline chunk-by-chunk ---
                for c0 in range(0, Q, 1024):
                    cw = min(1024, Q - c0)
                    sl = slice(c0, c0 + cw)
                    for mt, (wmt, zsb, bvec, svec) in enumerate(
                            ((w0s, zsb0, b0s, None), (w1s, zsb1, b1s, s1s))):
                        zp = zpsum.tile([128, 1024], f32, tag="zp")
                        for wo in range(0, cw, 512):
                            ww = min(512, cw - wo)
                            MM(zp[:, wo:wo + ww], wmt,
                               xh[:, c0 + wo:c0 + wo + ww],
                               start=True, stop=True)
                        ACT(zsb[:, sl], zp[:, 0:cw], AF.Sigmoid,
                            bias=bvec, scale=(svec if svec is not None
                                              else 1.0))
                    # tanh(j) = 2*sigma(2j) - 1  (in-place on zsb1 hi rows)
                    nc.vector.tensor_scalar(zsb1[64:128, sl], zsb1[64:128, sl],
                                            2.0, -1.0, ALU.mult, ALU.add)
                    # p = sigma(i) * tanh(j)   (hi partitions)
                    TT(ppair[64:128, sl], zsb0[64:128, sl], zsb1[64:128, sl],
                       ALU.mult)
                    # move p across the partition boundary
                    nc.sync.dma_start(ppair[0:64, sl], ppair[64:128, sl])
                    # c' = sigma(f)*c + p
                    TT(cfb[:, sl], zsb0[0:64, sl], cst[:, sl], ALU.mult)
                    TT(cst[:, sl], cfb[:, sl], ppair[0:64, sl], ALU.add)
                    # h = sigma(o) * tanh(c)
                    ACT(tcb[:, sl], cst[:, sl], AF.Tanh)
                    TT(xh_n[0:64, sl], zsb1[0:64, sl], tcb[:, sl], ALU.mult)

                # ================= stage 2 (step t) =================
                h = xh_n[0:64]
                h2r = h2prev
                z2a = z2psum.tile([128, 512], f32, tag="z2a")
                z2b = z2psum.tile([128, 512], f32, tag="z2b")
                # T0 = [i0|C0], T1 = [i1|C1] in z2a; T2 = [i2|C2], T3 = [f|o]
                MM(z2a[:, 0:BC], wy0s, h[:, 0:BC], start=True, stop=False)
                MM(z2a[:, 0:BC], r0s, h2r, start=False, stop=True)
                for s in range(NS):
                    MM(z2a[:, BC:2 * BC], wps[:, s * 128:(s + 1) * 128],
                       h[:, (1 + s) * BC:(2 + s) * BC],
                       start=(s == 0), stop=False)
                MM(z2a[:, BC:2 * BC], r1s, h2r, start=False, stop=True)
                for s in range(NS):
                    MM(z2b[:, 0:BC], wns[:, s * 128:(s + 1) * 128],
                       h[:, (11 + s) * BC:(12 + s) * BC],
                       start=(s == 0), stop=False)
                MM(z2b[:, 0:BC], r2s, h2r, start=False, stop=True)
                MM(z2b[:, BC:2 * BC], wys3s, h[:, 0:BC], start=True,
                   stop=False)
                MM(z2b[:, BC:2 * BC], r3s, h2r, start=False, stop=True)
                # g = tanh(c2 @ Wa)
                g_ps = mpsum.tile([64, BC], f32, tag="sm")
                MM(g_ps, was, c2, start=True, stop=True)
                gsb = l1pool.tile([64, BC], f32, tag="g")
                ACT(gsb, g_ps, AF.Tanh)
                # sigma passes  (C-gates: sigma(2x+2b) -> tanh fix later)
                z2sb = l2pool.tile([128, 1024], f32, tag="z2sb")
                ACT(z2sb[:, 0:256], z2a[:, 0:BC], AF.Sigmoid,
                    bias=bt0s, scale=sc02s)
                ACT(z2sb[:, 256:512], z2a[:, BC:2 * BC], AF.Sigmoid,
                    bias=bt1s, scale=sc02s)
                ACT(z2sb[:, 512:768], z2b[:, 0:BC], AF.Sigmoid,
                    bias=bt2s, scale=sc02s)
                ACT(z2sb[:, 768:1024], z2b[:, BC:2 * BC], AF.Sigmoid,
                    bias=bt3s)
                # move [C-gates sigma | sigma_o] to low partitions
                mv = l2pool.tile([64, 1024], f32, tag="mv")
                nc.sync.dma_start(mv, z2sb[64:128, :])
                nc.vector.tensor_scalar(mv[:, 0:768], mv[:, 0:768],
                                        2.0, -1.0, ALU.mult, ALU.add)
                # l_k = sigma(i_k) * tanh(C_k)
                lall = l2pool.tile([64, 768], f32, tag="lall")
                TT(lall, z2sb[0:64, 0:768], mv[:, 0:768], ALU.mult)
                # u_k = sum_h l_k * g
                lg = l2pool.tile([64, 768], f32r, tag="lg")
                for k in range(3):
                    TT(lg[:, k * BC:(k + 1) * BC],
                       lall[:, k * BC:(k + 1) * BC], gsb, ALU.mult)
                u_ps = mpsum.tile([3, BC], f32, tag="sm")
                for k in range(3):
                    MM(u_ps, onesels[:, 3 * k:3 * (k + 1)],
                       lg[:, k * BC:(k + 1) * BC],
                       start=(k == 0), stop=(k == 2))
                usb = l1pool.tile([3, BC], f32, tag="usb")
                nc.vector.tensor_copy(usb, u_ps)
                # transpose u -> [128, 2, 3]
                uT = l1pool.tile([128, 2, 3], f32, tag="uT")
                for bh in range(2):
                    tp = mpsum.tile([128, 3], f32, tag="sm")
                    nc.tensor.transpose(tp, usb[:, bh * 128:(bh + 1) * 128],
                                        id3s)
                    nc.vector.tensor_copy(uT[:, bh, :], tp)
                # softmax over k via sigma-ratio with max-subtract
                um = l1pool.tile([128, 2], f32, tag="um")
                nc.vector.tensor_reduce(um, uT[:, :, :], mybir.AxisListType.X,
                                        ALU.max)
                for k in range(3):
                    TT(uT[:, :, k], uT[:, :, k], um, ALU.subtract)
                ssb = l1pool.tile([128, 2, 3], f32, tag="ssb")
                ACT(ssb[:, :, :], uT[:, :, :], AF.Sigmoid)
                q1 = l1pool.tile([128, 2, 3], f32, tag="q1")
                nc.vector.tensor_scalar(q1[:, :, :], ssb[:, :, :], -1.0, 1.0,
                                        ALU.mult, ALU.add)
                nc.vector.reciprocal(q1[:, :, :], q1[:, :, :])
                wk = l1pool.tile([128, 2, 3], f32, tag="wk")
                TT(wk[:, :, :], ssb[:, :, :], q1[:, :, :], ALU.mult)
                ws = l1pool.tile([128, 2], f32, tag="ws")
                nc.vector.tensor_reduce(ws, wk[:, :, :], mybir.AxisListType.X,
                                        ALU.add)
                nc.vector.reciprocal(ws, ws)
                for k in range(3):
                    TT(wk[:, :, k], wk[:, :, k], ws, ALU.mult)
                # transpose a back -> [3, 256]
                asb = l1pool.tile([3, BC], f32r, tag="asb")
                for bh in range(2):
                    tp = mpsum.tile([3, 128], f32, tag="sm")
                    nc.tensor.transpose(tp, wk[:, bh, :], id128s)
                    nc.vector.tensor_copy(asb[:, bh * 128:(bh + 1) * 128], tp)
                # L = sum_k a_k * l_k  (broadcast a over 64 partitions via PE)
                a3 = l1pool.tile([1, 3 * BC], f32r, tag="a3")
                nc.sync.dma_start(a3, asb)   # [3,256] -> [1,768] row-major
                Lb = l1pool.tile([64, BC], f32, tag="Lb")
                Lt = l1pool.tile([64, BC], f32, tag="Lt")
                for k in range(3):
                    ab_ps = mpsum.tile([64, BC], f32, tag="sm")
                    MM(ab_ps, ones64s, a3[0:1, k * BC:(k + 1) * BC],
                       start=True, stop=True)
                    if k == 0:
                        TT(Lb, ab_ps, lall[:, 0:BC], ALU.mult)
                    else:
                        TT(Lt, ab_ps, lall[:, k * BC:(k + 1) * BC], ALU.mult)
                        TT(Lb, Lb, Lt, ALU.add)
                # c2' = sigma(f)*c2 + L ; h2 = sigma(o)*tanh(c2')
                t2 = l1pool.tile([64, BC], f32, tag="t2")
                TT(t2, z2sb[0:64, 768:1024], c2, ALU.mult)
                TT(c2, t2, Lb, ALU.add)
                tc2 = l1pool.tile([64, BC], f32, tag="tc2")
                ACT(tc2, c2, AF.Tanh)
                h2 = l1pool.tile([64, BC], f32r, tag="h2")
                TT(h2, mv[:, 768:1024], tc2, ALU.mult)
                # e accumulation + transposed H2 store
                MM(e_ps, wtsels[:, t * T:(t + 1) * T], h2,
                   start=(t == 0), stop=(t == T - 1))
                for bh in range(2):
                    tp = mpsum.tile([128, 64], f32, tag="sm")
                    nc.tensor.transpose(tp.bitcast(f32r),
                                        h2[:, bh * 128:(bh + 1) * 128],
                                        id64rs)
                    nc.vector.tensor_copy(H2T[:, bh, t, :], tp)
                h2prev = h2

            # ======================= head =======================
            esb = spool.tile([50, BC], f32)
            ACT(esb, e_ps, AF.Tanh, bias=btvs)
            eT = spool.tile([128, 2, 50], f32)
            for bh in range(2):
                tp = mpsum.tile([128, 50], f32, tag="sm")
                nc.tensor.transpose(tp, esb[:, bh * 128:(bh + 1) * 128],
                                    id50s)
                nc.vector.tensor_copy(eT[:, bh, :], tp)
            sh = spool.tile([128, 2, 50], f32)
            ACT(sh[:, :, :], eT[:, :, :], AF.Sigmoid)
            qh = spool.tile([128, 2, 50], f32)
            nc.vector.tensor_scalar(qh[:, :, :], sh[:, :, :], -1.0, 1.0,
                                    ALU.mult, ALU.add)
            nc.vector.reciprocal(qh[:, :, :], qh[:, :, :])
            whd = spool.tile([128, 2, 50], f32)
            TT(whd[:, :, :], sh[:, :, :], qh[:, :, :], ALU.mult)
            wsh = spool.tile([128, 2], f32)
            nc.vector.tensor_reduce(wsh, whd[:, :, :], mybir.AxisListType.X,
                                    ALU.add)
            nc.vector.reciprocal(wsh, wsh)
            # ctx: scale H2T rows by w and tree-reduce over t
            for bh in range(2):
                for tt in range(T):
                    nc.vector.tensor_scalar(
                        H2T[:, bh, tt, :], H2T[:, bh, tt, :],
                        whd[:, bh, tt:tt + 1], None, ALU.mult)
            # tree reduce 50 -> 1 (in place over t axis)
            n = T
            while n > 1:
                half = n // 2
                for bh in range(2):
                    TT(H2T[:, bh, 0:half, :], H2T[:, bh, 0:half, :],
                       H2T[:, bh, half:2 * half, :], ALU.add)
                if n % 2 == 1:
                    for bh in range(2):
                        TT(H2T[:, bh, 0, :], H2T[:, bh, 0, :],
                           H2T[:, bh, n - 1, :], ALU.add)
                n = half
            ctxT = spool.tile([128, 2, 64], f32)
            for bh in range(2):
                nc.vector.tensor_scalar(ctxT[:, bh, :], H2T[:, bh, 0, :],
                                        wsh[:, bh:bh + 1], None, ALU.mult)
            ctxs = spool.tile([64, BC], f32r)
            for bh in range(2):
                tp = mpsum.tile([64, 128], f32, tag="sm")
                nc.tensor.transpose(tp, ctxT[:, bh, :], id128s)
                nc.vector.tensor_copy(ctxs[:, bh * 128:(bh + 1) * 128], tp)
            r1_ps = mpsum.tile([64, BC], f32, tag="sm")
            MM(r1_ps, wd1s, ctxs, start=True, stop=True)
            r1sb = spool.tile([64, BC], f32r)
            ACT(r1sb, r1_ps, AF.Relu, bias=bd1s)
            o_ps = mpsum.tile([1, BC], f32, tag="sm")
            MM(o_ps, wd2s, r1sb, start=True, stop=True)
            osb = spool.tile([1, BC], f32)
            ACT(osb, o_ps, AF.Identity, bias=constss[0:1, 1:2])
            nc.sync.dma_start(out_d, osb)

    nc.compile()
    return nc


# ---------------------------------------------------------------------------
# Host-side input prep
# ---------------------------------------------------------------------------

def _prep_weights(K1, b1, Wc0, bc0, Wc1, bc1, Wc2, bc2,
                  Wi0, bi0, Wi1, bi1, Wi2, bi2, Wf, bf, Wo, bo, Wa,
                  Wt, bt, Wd1, bd1, Wd2, bd2):
    f32, f16 = np.float32, np.float16
    K1 = np.asarray(K1, f32); b1 = np.asarray(b1, f32)
    d = {}
    # stage1 lhsTs: rows = [Kh(64); Kx(1)]; col order tiles [f|i], [o|j]
    kr = np.concatenate([K1[1:65], K1[0:1]], axis=0)  # [65, 256]
    d["w0"] = np.concatenate([kr[:, 128:192], kr[:, 0:64]], axis=1).astype(f16)
    d["w1"] = np.concatenate([kr[:, 192:256], kr[:, 64:128]],
                             axis=1).astype(f16)
    d["bias0"] = np.concatenate([b1[128:192] + 1.0,
                                 b1[0:64]]).reshape(128, 1).astype(f32)
    d["bias1"] = np.concatenate([b1[192:256],
                                 2.0 * b1[64:128]]).reshape(128, 1).astype(f32)
    d["scale1"] = np.concatenate([np.ones(64), 2.0 * np.ones(64)]
                                 ).reshape(128, 1).astype(f32)
    # stage2
    Wi0, Wc0 = np.asarray(Wi0, f32), np.asarray(Wc0, f32)
    Wi1, Wc1 = np.asarray(Wi1, f32), np.asarray(Wc1, f32)
    Wi2, Wc2 = np.asarray(Wi2, f32), np.asarray(Wc2, f32)
    Wf_, Wo_ = np.asarray(Wf, f32), np.asarray(Wo, f32)
    d["wy0"] = np.concatenate([Wi0[:64], Wc0[:64]], axis=1).astype(f16)
    d["wys3"] = np.concatenate([Wf_[:64], Wo_[:64]], axis=1).astype(f16)
    wp = np.zeros((64, 1280), f16)
    wn = np.zeros((64, 1280), f16)
    for s in range(NS):
        wp[:, s * 128:s * 128 + 64] = Wi1[s * 64:(s + 1) * 64]
        wp[:, s * 128 + 64:(s + 1) * 128] = Wc1[s * 64:(s + 1) * 64]
        wn[:, s * 128:s * 128 + 64] = Wi2[s * 64:(s + 1) * 64]
        wn[:, s * 128 + 64:(s + 1) * 128] = Wc2[s * 64:(s + 1) * 64]
    d["wp"], d["wn"] = wp, wn
    d["r0"] = np.concatenate([Wi0[64:], Wc0[64:]], axis=1).astype(f32)
    d["r1"] = np.concatenate([Wi1[640:], Wc1[640:]], axis=1).astype(f32)
    d["r2"] = np.concatenate([Wi2[640:], Wc2[640:]], axis=1).astype(f32)
    d["r3"] = np.concatenate([Wf_[64:], Wo_[64:]], axis=1).astype(f32)
    d["wa"] = np.asarray(Wa, f32)
    bi0, bc0 = np.asarray(bi0, f32), np.asarray(bc0, f32)
    bi1, bc1 = np.asarray(bi1, f32), np.asarray(bc1, f32)
    bi2, bc2 = np.asarray(bi2, f32), np.asarray(bc2, f32)
    bf_, bo_ = np.asarray(bf, f32), np.asarray(bo, f32)
    d["bt0"] = np.concatenate([bi0, 2 * bc0]).reshape(128, 1).astype(f32)
    d["bt1"] = np.concatenate([bi1, 2 * bc1]).reshape(128, 1).astype(f32)
    d["bt2"] = np.concatenate([bi2, 2 * bc2]).reshape(128, 1).astype(f32)
    d["bt3"] = np.concatenate([bf_, bo_]).reshape(128, 1).astype(f32)
    d["sc02"] = d["scale1"]
    onesel = np.zeros((64, 9), f32)
    for k in range(3):
        onesel[:, 3 * k + k] = 1.0
    d["onesel"] = onesel
    d["ones1x64"] = np.ones((1, 64), f32)
    Wt = np.asarray(Wt, f32)
    wtsel = np.zeros((64, T * T), f32)
    for t_ in range(T):
        wtsel[:, t_ * T + t_] = Wt[:, 0]
    d["wtsel"] = wtsel
    d["id3"] = np.eye(3, dtype=f32)
    d["id50"] = np.eye(50, dtype=f32)
    d["id64"] = np.eye(64, dtype=f32)
    d["id64r"] = np.eye(64, dtype=f32)
    d["id128"] = np.eye(128, dtype=f32)
    d["wd1"] = np.asarray(Wd1, f32)
    d["bd1"] = np.asarray(bd1, f32).reshape(64, 1)
    d["wd2"] = np.asarray(Wd2, f32)
    d["zeros64"] = np.zeros((64, BC), f32)
    d["btv"] = np.full((50, 1), float(np.asarray(bt).reshape(-1)[0]), f32)
    d["consts"] = np.array([[float(np.asarray(bt).reshape(-1)[0]),
                             float(np.asarray(bd2).reshape(-1)[0])]], f32)
    return d


def _run_bass(Y, P, N, wd):
    global _LAST_HW_NS
    from concourse.bass_utils import run_bass_kernel_spmd
    if "nc" not in _CACHE:
        _CACHE["nc"] = _build()
    nc = _CACHE["nc"]
    series = np.concatenate([np.asarray(Y, np.float32),
                             np.asarray(P, np.float32),
                             np.asarray(N, np.float32)], axis=2)  # [B,T,21]
    in_maps = []
    for c in range(NCORES):
        blk = series[c * BC:(c + 1) * BC]            # [BC,T,21]
        xtc = np.ascontiguousarray(
            np.transpose(blk, (1, 2, 0)).reshape(T, Q)).astype(np.float16)
        m = {"xt": xtc}
        m.update(wd)
        in_maps.append(m)
    trace = os.environ.get("KERNEL_TRACE", "0") == "1"
    if trace:
        try:
            import antenv.axon_hooks  # noqa: F401
        except ImportError:
            trace = False
    kw = {}
    if trace:
        kw = {"trace": True, "trace_cores": [0]}
    res = run_bass_kernel_spmd(nc, in_maps, list(range(NCORES)), **kw)
    if getattr(res, "exec_time_ns", None) is not None:
        _LAST_HW_NS = res.exec_time_ns
    outs = [res.results[c]["out"].reshape(BC, 1) for c in range(NCORES)]
    return np.concatenate(outs, axis=0).astype(np.float32)


# ---------------------------------------------------------------------------
# Host fallback (numpy, used only if the device path fails)
# ---------------------------------------------------------------------------

def _host_ref(Y, P, N, K1, b1, Wc0, bc0, Wc1, bc1, Wc2, bc2,
              Wi0, bi0, Wi1, bi1, Wi2, bi2, Wf, bf, Wo, bo, Wa,
              Wt, bt, Wd1, bd1, Wd2, bd2):
    f32 = np.float32
    sig = lambda x: 1.0 / (1.0 + np.exp(-x))
    series = np.concatenate([Y, P, N], axis=2)
    x21 = np.moveaxis(series, 2, 0)[..., None].reshape(NSER * B, T, 1)
    Kx, Kh = K1[0], K1[1:]
    h = np.zeros((NSER * B, H), f32); c = np.zeros((NSER * B, H), f32)
    hs = np.empty((NSER * B, T, H), f32)
    for t in range(T):
        z = x21[:, t, 0:1] * Kx[None, :] + h @ Kh + b1
        i, j, f, o = np.split(z, 4, axis=1)
        c = sig(f + 1.0) * c + sig(i) * np.tanh(j)
        h = sig(o) * np.tanh(c)
        hs[:, t] = h
    hs = hs.reshape(NSER, B, T, H)
    Y1 = hs[0]
    pres = np.moveaxis(hs[1:1 + NS], 0, 2).reshape(B, T, NS * H)
    nres = np.moveaxis(hs[1 + NS:], 0, 2).reshape(B, T, NS * H)
    X = np.concatenate([Y1, pres, nres], axis=2)
    h2 = np.zeros((B, H), f32); c2 = np.zeros((B, H), f32)
    H2 = np.empty((B, T, H), f32)
    for t in range(T):
        xt_ = X[:, t]
        xY, xP, xN = xt_[:, :H], xt_[:, H:H + DP], xt_[:, H + DP:]
        def br(x_, Wi, bi_, Wc, bc_):
            z_ = np.concatenate([x_, h2], axis=1)
            return sig(z_ @ Wi + bi_) * np.tanh(z_ @ Wc + bc_)
        l0 = br(xY, Wi0, bi0, Wc0, bc0)
        l1 = br(xP, Wi1, bi1, Wc1, bc1)
        l2 = br(xN, Wi2, bi2, Wc2, bc2)
        g = np.tanh(c2 @ Wa)
        u = np.stack([(l0 * g).sum(1), (l1 * g).sum(1), (l2 * g).sum(1)], 1)
        u = u - u.max(1, keepdims=True)
        e_ = np.exp(u); a = e_ / e_.sum(1, keepdims=True)
        L = a[:, 0:1] * l0 + a[:, 1:2] * l1 + a[:, 2:3] * l2
        zY = np.concatenate([xY, h2], axis=1)
        c2 = sig(zY @ Wf + bf) * c2 + L
        h2 = sig(zY @ Wo + bo) * np.tanh(c2)
        H2[:, t] = h2
    e = np.tanh(H2 @ Wt + bt)
    e = e - e.max(1, keepdims=True)
    bta = np.exp(e); bta = bta / bta.sum(1, keepdims=True)
    ctx = (bta * H2).sum(1)
    r1 = np.maximum(ctx @ Wd1 + bd1, 0.0)
    return (r1 @ Wd2 + bd2).astype(f32)


# ---------------------------------------------------------------------------
# Entry point
# ---------------------------------------------------------------------------

def kernel(Y, P, N, K1, b1, Wc0, bc0, Wc1, bc1, Wc2, bc2,
           Wi0, bi0, Wi1, bi1, Wi2, bi2, Wf, bf, Wo, bo, Wa,
           Wt, bt, Wd1, bd1, Wd2, bd2):
    args = dict(K1=K1, b1=b1, Wc0=Wc0, bc0=bc0, Wc1=Wc1, bc1=bc1, Wc2=Wc2,
                bc2=bc2, Wi0=Wi0, bi0=bi0, Wi1=Wi1, bi1=bi1, Wi2=Wi2,
                bi2=bi2, Wf=Wf, bf=bf, Wo=Wo, bo=bo, Wa=Wa, Wt=Wt, bt=bt,
                Wd1=Wd1, bd1=bd1, Wd2=Wd2, bd2=bd2)
    if os.environ.get("KERNEL_NO_BASS", "0") != "1":
        try:
            wd = _prep_weights(**args)
            return _run_bass(Y, P, N, wd)
        except Exception as e:  # noqa: BLE001
            import traceback
            traceback.print_exc()
            sys.stderr.write(f"bass path failed ({e!r}); host fallback\n")
    f32 = np.float32
    return _host_ref(np.asarray(Y, f32), np.asarray(P, f32),
                     np.asarray(N, f32),
                     *[np.asarray(args[k], f32) for k in
                       ("K1", "b1", "Wc0", "bc0", "Wc1", "bc1", "Wc2", "bc2",
                        "Wi0", "bi0", "Wi1", "bi1", "Wi2", "bi2", "Wf", "bf",
                        "Wo", "bo", "Wa", "Wt", "bt", "Wd1", "bd1", "Wd2",
                        "bd2")])
